# revision 5
# baseline (speedup 1.0000x reference)
"""AttnBlock (GroupNorm + spatial self-attention + residual) on 8 trn2 NeuronCores.

Sharding: 8 cores = 2 batches x 4 query-chunks of 1024 spatial positions.
Each core receives x[b] rolled so its query range is columns [0, 1024); all
cores run one identical SPMD program.

Host-side algebra (exact up to dropped softmax-invariant terms):
  scores^T[j,i] = x[:,j] . (M x[:,i] + bq2)  with M = diag(A) Wqk diag(A),
    Wqk = C^-1/2 wk^T wq, bq2 = A*(Wqk Bv + C^-1/2 wk^T bq); A/Bv are the
    per-(batch,channel) GroupNorm affine folded on host (hn = A*x + Bv).
  out = x + (sum_j es_j (WovA x_j)) / (sum_j es_j) + bovE  with
    WovA = wo wv diag(A), bovE = Wov Bv + wo bv + bo.

Device side is pure fp8 matmul pipeline: scores / V-projection / attention
all run as fp8e4 DoubleRow matmuls (2 K-tiles packed along the free dim);
exp runs on the ACT engine with a -EOFF bias so es fits fp8 range (the
offset cancels in the softmax ratio).
"""

import ml_dtypes
import numpy as np

import concourse.bass as bass
import concourse.tile as tile
from concourse import bacc, mybir
from concourse import bass_utils

F32 = mybir.dt.float32
F32R = mybir.dt.float32r
F8 = mybir.dt.float8e4

B, C, D, H, W = 2, 512, 4, 32, 32
L = D * H * W            # 4096
G = 32                   # groupnorm groups
EPS = 1e-6
P = 128
NT = C // P              # 4 channel tiles
NT2 = NT // 2            # 2 channel pairs (DoubleRow)
NJ = L // P              # 32 key tiles
NJ2 = NJ // 2            # 16 key pairs
IC = 512                 # query-chunk width
LQ = 1024                # query cols per core
NIC = LQ // IC           # 2 i-chunks
NCORES = 8
DEPTH = 2                # attention pipeline depth, in key PAIRS
EOFF = 4.0               # exp offset: es = exp(s - EOFF), cancels in softmax

_CACHE = {}


def _build():
    nc = bacc.Bacc(trn_type="TRN2", target_bir_lowering=False, debug=False,
                   num_devices=NCORES)
    x8_d = nc.dram_tensor("x8", [NT, P, L], F8, kind="ExternalInput").ap()
    xf_d = nc.dram_tensor("xf", [NT, P, LQ], F32R, kind="ExternalInput").ap()
    wqk_d = nc.dram_tensor("wqkT", [C, C], F32R, kind="ExternalInput").ap()
    wov_d = nc.dram_tensor("wovT", [C, C], F8, kind="ExternalInput").ap()
    bq2_d = nc.dram_tensor("bq2", [C], F32, kind="ExternalInput").ap()
    bov_d = nc.dram_tensor("bovE", [C], F32, kind="ExternalInput").ap()
    ones_d = nc.dram_tensor("ones8", [P, 2 * P], F8, kind="ExternalInput").ap()
    onesr_d = nc.dram_tensor("onesr", [1, P], F32R, kind="ExternalInput").ap()
    out_d = nc.dram_tensor("out", [C, LQ], F32, kind="ExternalOutput").ap()

    DR = mybir.MatmulPerfMode.DoubleRow

    with tile.TileContext(nc) as tc:
        with (
            tc.tile_pool(name="big", bufs=1) as big,
            tc.tile_pool(name="small", bufs=1) as small,
            tc.tile_pool(name="est", bufs=DEPTH + 3) as est,
            tc.tile_pool(name="osb", bufs=3) as osb,
            tc.tile_pool(name="zp", bufs=6) as zp,
            tc.tile_pool(name="tmp", bufs=4) as tmp,
            tc.tile_pool(name="ps", bufs=3, space="PSUM") as ps,
            tc.tile_pool(name="pho", bufs=4, space="PSUM") as pho,
            tc.tile_pool(name="psum1", bufs=1, space="PSUM") as psum1,
        ):
            # ---- DMA in. sync queue: qk-proj deps first, then keys-side x8.
            # scalar queue: wov8; gpsimd queue: small consts. ----
            wqk = big.tile([P, NT, C], F32R, tag="wqk")
            nc.sync.dma_start(wqk[:], wqk_d.rearrange("(t p) c -> p t c", p=P))
            xf = big.tile([P, NT, LQ], F32R, tag="xf")
            xf_r = xf_d.rearrange("t p l -> p t l")
            for t in range(NT):
                nc.sync.dma_start(xf[:, t, :], xf_r[:, t, :])
            xt = big.tile([P, NT, L], F8, tag="xt")
            for t in range(NT):
                nc.sync.dma_start(xt[:, t, :], x8_d[t])
            wov = small.tile([P, NT, C], F8, tag="wov")
            nc.scalar.dma_start(wov[:], wov_d.rearrange("(t p) c -> p t c", p=P))
            bq2 = small.tile([P, NT], F32, tag="bq2")
            nc.gpsimd.dma_start(bq2[:], bq2_d.rearrange("(t p) -> p t", p=P))
            bov = small.tile([P, NT], F32, tag="bov")
            nc.gpsimd.dma_start(bov[:], bov_d.rearrange("(t p) -> p t", p=P))

            ones8 = small.tile([P, 2, P], F8, tag="ones8")
            nc.gpsimd.dma_start(ones8[:], ones_d.rearrange("p (two q) -> p two q", two=2))
            onesr = small.tile([1, P], F32R, tag="onesr")
            nc.gpsimd.dma_start(onesr[:], onesr_d)
            beoff = small.tile([P, 1], F32, tag="beoff")
            nc.vector.memset(beoff[:], -EOFF)

            # preload the Exp table while DMA streams in
            dum = tmp.tile([P, 1], F32, tag="dum")
            nc.scalar.activation(dum[:], beoff[:], mybir.ActivationFunctionType.Exp)

            # ---- qk projection: qk8[:, i] = fp8(M x_i + bq2), fp32r matmul ----
            qk = big.tile([P, NT, LQ], F8, tag="qk")
            for icn in range(NIC):
                for tq in range(NT):
                    qps = ps.tile([P, IC], F32, tag="mm")
                    for t in range(NT):
                        nc.tensor.matmul(qps[:], wqk[:, t, bass.ts(tq, P)],
                                         xf[:, t, bass.ts(icn, IC)],
                                         start=(t == 0), stop=(t == NT - 1))
                    nc.vector.tensor_tensor(qk[:, tq, bass.ts(icn, IC)], qps[:],
                                            bq2[:, tq:tq + 1].to_broadcast((P, IC)),
                                            mybir.AluOpType.add)

            # ---- z[t][icn] = x_residual + bovE on GpSimd (idle here) ----
            zall = {}
            for icn in range(NIC):
                for t in range(NT):
                    z = zp.tile([P, IC], F32, tag="zp", name=f"z{icn}_{t}")
                    nc.gpsimd.tensor_tensor(z[:], xf[:, t, bass.ts(icn, IC)].bitcast(F32),
                                            bov[:, t:t + 1].to_broadcast((P, IC)),
                                            mybir.AluOpType.add)
                    zall[(icn, t)] = z

            # ---- voT projection (fp8 DoubleRow): vot[j, c] = (WovA x)[c, j]^T
            # packed as [P, jj2, half, c] for DoubleRow attention rhs ----
            vot = big.tile([P, NJ2, 2, C], F8, tag="vot")
            for j in range(NJ):
                vps = ps.tile([P, C], F32, tag="mm")
                for h in range(NT2):
                    nc.tensor.matmul(vps[:], xt[:, 2 * h:2 * h + 2, bass.ts(j, P)],
                                     wov[:, 2 * h:2 * h + 2, :],
                                     start=(h == 0), stop=(h == NT2 - 1),
                                     perf_mode=DR)
                nc.vector.tensor_copy(vot[:, j // 2, j % 2, :], vps[:])

            # ---- attention per i-chunk ----
            pending_fin = [None]

            def make_finalize(icn, sums, hops):
                def fin():
                    zs = [zall[(icn, t)] for t in range(NT)]
                    rec = small.tile([1, IC], F32R, tag=f"rec{icn}",
                                     name=f"rec{icn}")
                    with nc.allow_low_precision(reason="fp32r denom ~1e-4"):
                        nc.vector.reciprocal(rec[:], sums[0:1, :])
                    rps = ps.tile([P, IC], F32, tag="mm", name=f"rps{icn}")
                    nc.tensor.matmul(rps[:], onesr[:], rec[:], start=True, stop=True)
                    rbc = tmp.tile([P, IC], F32, tag="rbc", name=f"rbc{icn}")
                    nc.vector.tensor_copy(rbc[:], rps[:])
                    last = icn == NIC - 1
                    for t in range(NT):
                        o = osb.tile([P, IC], F32, tag="osb", name=f"o{icn}_{t}")
                        nc.vector.tensor_tensor(o[:], hops[t][:], rbc[:],
                                                mybir.AluOpType.mult)
                        eng = nc.gpsimd if (last and t >= 2) else nc.vector
                        eng.tensor_tensor(o[:], o[:], zs[t][:],
                                          mybir.AluOpType.add)
                        nc.sync.dma_start(out_d[bass.ts(t, P), bass.ts(icn, IC)], o[:])
                return fin

            for icn in range(NIC):
                sums = psum1.tile([P, IC], F32, tag="sums", name=f"sums{icn}")
                hops = [pho.tile([P, IC], F32, tag="ho", name=f"ho_{icn}_{t}")
                        for t in range(NT)]
                ests = [None] * NJ2

                def consume(kk, sums=sums, hops=hops, ests=ests):
                    es2 = ests[kk]
                    nc.tensor.matmul(sums[:], ones8[:], es2[:],
                                     start=(kk == 0), stop=(kk == NJ2 - 1),
                                     perf_mode=DR)
                    for t in range(NT):
                        nc.tensor.matmul(hops[t][:], vot[:, kk, :, bass.ts(t, P)],
                                         es2[:],
                                         start=(kk == 0), stop=(kk == NJ2 - 1),
                                         perf_mode=DR)
                    ests[kk] = None

                for j in range(NJ):
                    if j == 2 and pending_fin[0] is not None:
                        pending_fin[0]()
                        pending_fin[0] = None
                    kk, half = divmod(j, 2)
                    sps = ps.tile([P, IC], F32, tag="mm", name=f"sps{icn}_{j}")
                    for h in range(NT2):
                        nc.tensor.matmul(sps[:], xt[:, 2 * h:2 * h + 2, bass.ts(j, P)],
                                         qk[:, 2 * h:2 * h + 2, bass.ts(icn, IC)],
                                         start=(h == 0), stop=(h == NT2 - 1),
                                         perf_mode=DR)
                    if half == 0:
                        es2 = est.tile([P, 2, IC], F8, tag="est",
                                       name=f"est{icn}_{kk}")
                        ests[kk] = es2
                    nc.scalar.activation(ests[kk][:, half, :], sps[:],
                                         mybir.ActivationFunctionType.Exp,
                                         bias=beoff[:], scale=1.0)
                    if half == 1 and kk >= DEPTH:
                        consume(kk - DEPTH)
                for kk in range(NJ2 - DEPTH, NJ2):
                    consume(kk)
                pending_fin[0] = make_finalize(icn, sums, hops)
            pending_fin[0]()

    nc.compile()
    return nc


def _prep(inputs):
    s = float(C) ** -0.5
    wq = np.asarray(inputs["wq"], np.float64)
    wk = np.asarray(inputs["wk"], np.float64)
    wv = np.asarray(inputs["wv"], np.float64)
    wo = np.asarray(inputs["wo"], np.float64)
    bq = np.asarray(inputs["bq"], np.float64)
    bv = np.asarray(inputs["bv"], np.float64)
    bo = np.asarray(inputs["bo"], np.float64)
    gamma = np.asarray(inputs["gamma"], np.float64)
    beta = np.asarray(inputs["beta"], np.float64)
    Wqk = (wk.T @ wq) * s
    Wov = wo @ wv
    bqkv = (wk.T @ bq) * s
    bovv = wo @ bv + bo

    x = np.asarray(inputs["x"], np.float64).reshape(B, C, L)
    per_batch = []
    for b in range(B):
        xb = x[b]
        xg = xb.reshape(G, -1)
        mu = xg.mean(axis=1)
        var = xg.var(axis=1)
        rstd = 1.0 / np.sqrt(var + EPS)
        A = (gamma.reshape(G, -1) * rstd[:, None]).reshape(C)
        Bv = (beta.reshape(G, -1) - (gamma.reshape(G, -1) * (mu * rstd)[:, None])).reshape(C)
        M = A[:, None] * Wqk * A[None, :]
        bq2 = A * (Wqk @ Bv + bqkv)
        WovA = Wov * A[None, :]
        bovE = Wov @ Bv + bovv
        per_batch.append({
            "wqkT": np.ascontiguousarray(M.T, np.float32),      # [c_in, c_out]
            "wovT": np.ascontiguousarray(WovA.T).astype(ml_dtypes.float8_e4m3fn),
            "bq2": bq2.astype(np.float32),
            "bovE": bovE.astype(np.float32),
            "ones8": np.ones((P, 2 * P), ml_dtypes.float8_e4m3fn),
            "onesr": np.ones((1, P), np.float32),
        })
    return per_batch, x


LAST_RESULTS = None


def kernel(**inputs) -> np.ndarray:
    global LAST_RESULTS
    if "nc" not in _CACHE:
        _CACHE["nc"] = _build()
    nc = _CACHE["nc"]
    per_batch, x = _prep(inputs)
    in_maps = []
    for core in range(NCORES):
        b, chunk = divmod(core, 4)
        xr = np.roll(x[b], -LQ * chunk, axis=1)
        x8 = np.ascontiguousarray(xr.reshape(NT, P, L)).astype(ml_dtypes.float8_e4m3fn)
        xf = np.ascontiguousarray(xr[:, :LQ].reshape(NT, P, LQ)).astype(np.float32)
        in_maps.append({"x8": x8, "xf": xf, **per_batch[b]})
    res = bass_utils.run_bass_kernel_spmd(nc, in_maps, core_ids=list(range(NCORES)))
    LAST_RESULTS = res
    out = np.empty((B, C, L), np.float32)
    for core in range(NCORES):
        b, chunk = divmod(core, 4)
        out[b][:, LQ * chunk:LQ * (chunk + 1)] = res.results[core]["out"]
    return out.reshape(B, C, D, H, W)


# revision 15
# speedup vs baseline: 1.2978x; 1.2978x over previous
"""AttnBlock (GroupNorm + spatial self-attention + residual) on 8 trn2 NeuronCores.

Sharding: 8 cores = 2 batches x 4 query-chunks of 1024 spatial positions.
Each core receives x[b] rolled so its query range is columns [0, 1024); all
cores run one identical SPMD program.

Host-side algebra (exact up to dropped softmax-invariant terms):
  scores^T[j,i] = x[:,j] . (M x[:,i] + bq2)  with M = diag(A) Wqk diag(A),
    Wqk = C^-1/2 wk^T wq, bq2 = A*(Wqk Bv + C^-1/2 wk^T bq); A/Bv are the
    per-(batch,channel) GroupNorm affine folded on host (hn = A*x + Bv).
  out = x + (sum_j es_j (WovA x_j)) / (sum_j es_j) + bovE  with
    WovA = wo wv diag(A), bovE = Wov Bv + wo bv + bo.

Device pipeline (fp8e4 DoubleRow matmuls = 2 K-tiles packed per free dim):
  phase V: vot[j,:] = (WovA x_j)^T, fp8 DoubleRow, PSUM->SBUF casts split
    over DVE/ACT so the PE never waits on one engine.
  phase Q: qk = fp8(M x + bq2): fp32r matmuls (fp8 here pushes rel err past
    the gate), bias-fold via ACT Identity-with-bias on the PSUM read.
  attention: per 512-col query chunk, for each key pair: 2 DoubleRow score
    matmuls -> exp(s-EOFF) (ACT; 8 per chunk on DVE via the Schraudolph
    int trick, which lands within fp8 es rounding) -> DoubleRow sums/ho.
  EOFF keeps exp in fp8 range and cancels in the softmax ratio.

All inputs are host-pre-arranged to their SBUF layouts so every DMA is a
single contiguous 2D span.
"""

import ml_dtypes
import numpy as np

import concourse.bass as bass
import concourse.tile as tile
from concourse import bacc, mybir
from concourse import bass_utils

F32 = mybir.dt.float32
F32R = mybir.dt.float32r
F8 = mybir.dt.float8e4
I32 = mybir.dt.int32

B, C, D, H, W = 2, 512, 4, 32, 32
L = D * H * W            # 4096
G = 32                   # groupnorm groups
EPS = 1e-6
P = 128
NT = C // P              # 4 channel tiles
NT2 = NT // 2            # 2 channel pairs (DoubleRow)
NJ = L // P              # 32 key tiles
NJ2 = NJ // 2            # 16 key pairs
NQ = 4                   # x8 DMA quarter-blocks
IC = 512                 # query-chunk width
LQ = 1024                # query cols per core
NIC = LQ // IC           # 2 i-chunks
NCORES = 8
DEPTH = 3                # attention pipeline depth, in key PAIRS
EOFF = 4.0               # exp offset: es = exp(s - EOFF), cancels in softmax
A32C = 12102203.161561485            # 2^23 * log2(e)
B32C = 127.0 * 2 ** 23 - 366393.0 - EOFF * A32C
DVE_EXP_J = frozenset((3, 7, 11, 15, 19, 23, 27))

_CACHE = {}


def _build():
    nc = bacc.Bacc(trn_type="TRN2", target_bir_lowering=False, debug=False,
                   num_devices=NCORES)
    x8_d = nc.dram_tensor("x8", [NQ, P, NT, L // NQ], F8, kind="ExternalInput").ap()
    xf_d = nc.dram_tensor("xf", [P, NT, LQ], F32R, kind="ExternalInput").ap()
    wqk_d = nc.dram_tensor("wqkT", [P, NT, C], F32R, kind="ExternalInput").ap()
    wov_d = nc.dram_tensor("wovT", [P, NT, C], F8, kind="ExternalInput").ap()
    bq2_d = nc.dram_tensor("bq2", [P, NT], F32, kind="ExternalInput").ap()
    bov_d = nc.dram_tensor("bovE", [P, NT], F32, kind="ExternalInput").ap()
    ones_d = nc.dram_tensor("ones8", [P, 2, P], F8, kind="ExternalInput").ap()
    onesr_d = nc.dram_tensor("onesr", [1, P], F32R, kind="ExternalInput").ap()
    out_d = nc.dram_tensor("out", [C, LQ], F32, kind="ExternalOutput").ap()

    DR = mybir.MatmulPerfMode.DoubleRow
    LQ4 = L // NQ

    with tile.TileContext(nc) as tc:
        with (
            tc.tile_pool(name="big", bufs=1) as big,
            tc.tile_pool(name="small", bufs=1) as small,
            tc.tile_pool(name="est", bufs=DEPTH + 3) as est,
            tc.tile_pool(name="osb", bufs=6) as osb,
            tc.tile_pool(name="zp", bufs=6) as zp,
            tc.tile_pool(name="tmp", bufs=4) as tmp,
            tc.tile_pool(name="ps", bufs=3, space="PSUM") as ps,
            tc.tile_pool(name="pho", bufs=4, space="PSUM") as pho,
            tc.tile_pool(name="psum1", bufs=1, space="PSUM") as psum1,
        ):
            # ---- DMA in. All big inputs share one queue, ordered by first
            # use (the 16 DMA engines are one shared bandwidth pool, so
            # cross-queue parallelism only reorders completion): wov -> x8
            # quarters -> wqk -> xf. gpsimd SWDGE: small consts. ----
            wov = small.tile([P, NT, C], F8, tag="wov")
            nc.scalar.dma_start(wov[:], wov_d)
            xt = big.tile([P, NT, L], F8, tag="xt")
            for q in range(NQ):
                nc.scalar.dma_start(xt[:, :, bass.ts(q, LQ4)], x8_d[q])
            wqk = big.tile([P, NT, C], F32R, tag="wqk")
            nc.scalar.dma_start(wqk[:], wqk_d)
            xf = big.tile([P, NT, LQ], F32R, tag="xf")
            nc.scalar.dma_start(xf[:], xf_d)
            bq2 = small.tile([P, NT], F32, tag="bq2")
            nc.gpsimd.dma_start(bq2[:], bq2_d)
            bov = small.tile([P, NT], F32, tag="bov")
            nc.gpsimd.dma_start(bov[:], bov_d)
            ones8 = small.tile([P, 2, P], F8, tag="ones8")
            nc.gpsimd.dma_start(ones8[:], ones_d)
            onesr = small.tile([1, P], F32R, tag="onesr")
            nc.gpsimd.dma_start(onesr[:], onesr_d)
            beoff = small.tile([P, 1], F32, tag="beoff")
            nc.vector.memset(beoff[:], -EOFF)

            # preload the Exp table while DMA streams in
            dum = tmp.tile([P, 1], F32, tag="dum")
            nc.scalar.activation(dum[:], beoff[:], mybir.ActivationFunctionType.Exp)

            # ---- phase V: vot[j, c] = (WovA x)[c, j]^T, fp8 DoubleRow,
            # packed [P, jj2, half, c] for the attention-consume rhs ----
            vot = big.tile([P, NJ2, 2, C], F8, tag="vot")
            for j in range(NJ):
                vps = ps.tile([P, C], F32, tag="mm")
                for hh in range(NT2):
                    nc.tensor.matmul(vps[:], xt[:, 2 * hh:2 * hh + 2, bass.ts(j, P)],
                                     wov[:, 2 * hh:2 * hh + 2, :],
                                     start=(hh == 0), stop=(hh == NT2 - 1),
                                     perf_mode=DR)
                if j % 2 == 1:
                    nc.scalar.copy(vot[:, j // 2, j % 2, :], vps[:])
                else:
                    nc.vector.tensor_copy(vot[:, j // 2, j % 2, :], vps[:])

            # ---- phase Q: qk8 = fp8(M x + bq2), fp32r matmuls; bias fold on
            # the ACT PSUM->SBUF read ----
            qk = big.tile([P, NT, LQ], F8, tag="qk")
            for icn in range(NIC):
                for tq in range(NT):
                    qps = ps.tile([P, IC], F32, tag="mm")
                    for t in range(NT):
                        nc.tensor.matmul(qps[:], wqk[:, t, bass.ts(tq, P)],
                                         xf[:, t, bass.ts(icn, IC)],
                                         start=(t == 0), stop=(t == NT - 1))
                    nc.vector.tensor_tensor(qk[:, tq, bass.ts(icn, IC)], qps[:],
                                            bq2[:, tq:tq + 1].to_broadcast((P, IC)),
                                            mybir.AluOpType.add)

            # ---- z[t][icn] = x_residual + bovE on GpSimd (idle here) ----
            zall = {}
            for icn in range(NIC):
                for t in range(NT):
                    z = zp.tile([P, IC], F32, tag="zp", name=f"z{icn}_{t}")
                    nc.gpsimd.tensor_tensor(z[:], xf[:, t, bass.ts(icn, IC)].bitcast(F32),
                                            bov[:, t:t + 1].to_broadcast((P, IC)),
                                            mybir.AluOpType.add)
                    zall[(icn, t)] = z

            # ---- attention per i-chunk ----
            pending_fin = [None]

            def make_finalize(icn, hops, rbc):
                def fin():
                    zs = [zall[(icn, t)] for t in range(NT)]
                    # mult reads PSUM -> DVE only; adds (SBUF) split DVE/Pool
                    for t in range(NT):
                        o = osb.tile([P, IC], F32, tag="osb", name=f"o{icn}_{t}")
                        nc.vector.tensor_tensor(o[:], hops[t][:], rbc[:],
                                                mybir.AluOpType.mult)
                        eng = nc.vector if t < 2 else nc.gpsimd
                        eng.tensor_tensor(o[:], o[:], zs[t][:],
                                          mybir.AluOpType.add)
                        nc.sync.dma_start(out_d[bass.ts(t, P), bass.ts(icn, IC)], o[:])
                return fin

            for icn in range(NIC):
                sums = psum1.tile([P, IC], F32, tag="sums", name=f"sums{icn}")
                hops = [pho.tile([P, IC], F32, tag="ho", name=f"ho_{icn}_{t}")
                        for t in range(NT)]
                ests = [None] * NJ2
                fin_pre = {}

                def consume(kk, icn=icn, sums=sums, hops=hops, ests=ests,
                            fin_pre=fin_pre):
                    es2 = ests[kk]
                    last = kk == NJ2 - 1
                    nc.tensor.matmul(sums[:], ones8[:], es2[:],
                                     start=(kk == 0), stop=last,
                                     perf_mode=DR)
                    if last:
                        # normalizer chain overlaps the last 4 ho matmuls:
                        # recip (DVE) -> broadcast rps (PE) -> rbc (ACT)
                        rec = small.tile([1, IC], F32R, tag=f"rec{icn}",
                                         name=f"rec{icn}")
                        with nc.allow_low_precision(reason="fp32r denom ~1e-4"):
                            nc.vector.reciprocal(rec[:], sums[0:1, :])
                        rps = ps.tile([P, IC], F32, tag="mm", name=f"rps{icn}")
                        nc.tensor.matmul(rps[:], onesr[:], rec[:],
                                         start=True, stop=True)
                        rbc = tmp.tile([P, IC], F32, tag="rbc", name=f"rbc{icn}")
                        nc.scalar.copy(rbc[:], rps[:])
                        fin_pre["rbc"] = rbc
                    for t in range(NT):
                        nc.tensor.matmul(hops[t][:], vot[:, kk, :, bass.ts(t, P)],
                                         es2[:],
                                         start=(kk == 0), stop=last,
                                         perf_mode=DR)
                    ests[kk] = None

                for j in range(NJ):
                    if j == 2 and pending_fin[0] is not None:
                        pending_fin[0]()
                        pending_fin[0] = None
                    kk, half = divmod(j, 2)
                    sps = ps.tile([P, IC], F32, tag="mm", name=f"sps{icn}_{j}")
                    for hh in range(NT2):
                        nc.tensor.matmul(sps[:], xt[:, 2 * hh:2 * hh + 2, bass.ts(j, P)],
                                         qk[:, 2 * hh:2 * hh + 2, bass.ts(icn, IC)],
                                         start=(hh == 0), stop=(hh == NT2 - 1),
                                         perf_mode=DR)
                    if half == 0:
                        es2 = est.tile([P, 2, IC], F8, tag="est",
                                       name=f"est{icn}_{kk}")
                        ests[kk] = es2
                    if j in DVE_EXP_J:
                        yi = tmp.tile([P, IC], I32, tag="yi", name=f"yi{icn}_{j}")
                        nc.vector.tensor_scalar(yi[:], sps[:], A32C, B32C,
                                                mybir.AluOpType.mult,
                                                mybir.AluOpType.add)
                        nc.vector.tensor_scalar(ests[kk][:, half, :],
                                                yi[:].bitcast(F32), 0.0, None,
                                                mybir.AluOpType.max)
                    else:
                        nc.scalar.activation(ests[kk][:, half, :], sps[:],
                                             mybir.ActivationFunctionType.Exp,
                                             bias=beoff[:], scale=1.0)
                    if half == 1 and kk >= DEPTH:
                        consume(kk - DEPTH)
                for kk in range(NJ2 - DEPTH, NJ2):
                    consume(kk)
                pending_fin[0] = make_finalize(icn, hops, fin_pre["rbc"])
            pending_fin[0]()

    nc.compile()
    return nc


def _prep(inputs):
    s = float(C) ** -0.5
    wq = np.asarray(inputs["wq"], np.float64)
    wk = np.asarray(inputs["wk"], np.float64)
    wv = np.asarray(inputs["wv"], np.float64)
    wo = np.asarray(inputs["wo"], np.float64)
    bq = np.asarray(inputs["bq"], np.float64)
    bv = np.asarray(inputs["bv"], np.float64)
    bo = np.asarray(inputs["bo"], np.float64)
    gamma = np.asarray(inputs["gamma"], np.float64)
    beta = np.asarray(inputs["beta"], np.float64)
    Wqk = (wk.T @ wq) * s
    Wov = wo @ wv
    bqkv = (wk.T @ bq) * s
    bovv = wo @ bv + bo

    x = np.asarray(inputs["x"], np.float64).reshape(B, C, L)
    per_batch = []
    for b in range(B):
        xb = x[b]
        xg = xb.reshape(G, -1)
        mu = xg.mean(axis=1)
        var = xg.var(axis=1)
        rstd = 1.0 / np.sqrt(var + EPS)
        A = (gamma.reshape(G, -1) * rstd[:, None]).reshape(C)
        Bv = (beta.reshape(G, -1) - (gamma.reshape(G, -1) * (mu * rstd)[:, None])).reshape(C)
        M = A[:, None] * Wqk * A[None, :]
        bq2 = A * (Wqk @ Bv + bqkv)
        WovA = Wov * A[None, :]
        bovE = Wov @ Bv + bovv
        per_batch.append({
            # lhsT layouts [c_in, c_out] pre-blocked to SBUF [P, NT, C]
            "wqkT": np.ascontiguousarray(
                M.T.reshape(NT, P, C).swapaxes(0, 1), np.float32),
            "wovT": np.ascontiguousarray(
                WovA.T.reshape(NT, P, C).swapaxes(0, 1)).astype(ml_dtypes.float8_e4m3fn),
            "bq2": np.ascontiguousarray(
                bq2.reshape(NT, P).T, np.float32),
            "bovE": np.ascontiguousarray(
                bovE.reshape(NT, P).T, np.float32),
            "ones8": np.ones((P, 2, P), ml_dtypes.float8_e4m3fn),
            "onesr": np.ones((1, P), np.float32),
        })
    return per_batch, x


LAST_RESULTS = None


def kernel(**inputs) -> np.ndarray:
    global LAST_RESULTS
    if "nc" not in _CACHE:
        _CACHE["nc"] = _build()
    nc = _CACHE["nc"]
    per_batch, x = _prep(inputs)
    NQ4 = L // NQ
    in_maps = []
    for core in range(NCORES):
        b, chunk = divmod(core, 4)
        xr = np.roll(x[b], -LQ * chunk, axis=1)
        # x8: [NQ, P, NT, L//NQ] so each quarter is one contiguous DMA
        x8 = np.ascontiguousarray(
            xr.reshape(NT, P, NQ, NQ4).transpose(2, 1, 0, 3)).astype(ml_dtypes.float8_e4m3fn)
        xf = np.ascontiguousarray(
            xr[:, :LQ].reshape(NT, P, LQ).swapaxes(0, 1)).astype(np.float32)
        in_maps.append({"x8": x8, "xf": xf, **per_batch[b]})
    res = bass_utils.run_bass_kernel_spmd(nc, in_maps, core_ids=list(range(NCORES)))
    LAST_RESULTS = res
    out = np.empty((B, C, L), np.float32)
    for core in range(NCORES):
        b, chunk = divmod(core, 4)
        out[b][:, LQ * chunk:LQ * (chunk + 1)] = res.results[core]["out"]
    return out.reshape(B, C, D, H, W)
